# revision 6
# baseline (speedup 1.0000x reference)
"""NonLocal block kernel for 8 Trainium2 NeuronCores.

Algebraic restructuring: the softmax-free attention

    s = theta^T phi / N ;  y = s . g^T   (per batch)

is reassociated as y = (G/N) @ theta with G[i,j] = sum_m g[i,m] phi[j,m]
(a [32,32] matrix per batch).  Folding the surrounding 1x1 convs:

    out = (I + W_w (G/N) theta_w) @ target + (W_w (G/N) theta_b + W_b)

so after G is known the whole module is one 64x64 1x1-conv over target.

Sharding: batch b -> core pair (2b, 2b+1); each core of the pair computes
G for its batch redundantly (reads full ref/ref_align for the batch) and
produces half of the spatial output (no cross-core communication).

Precision: tolerance is 2e-2 rel; the G path only perturbs the output at
the ~1e-3 level, so refs and the phi/g conv run in fp8e4 (weights scaled
x16 to dodge denormals; the x256 on G is divided out once).  target, the
final conv, and the output are bf16 (~0.4%% rel), PSUM accumulation f32.

Device layouts (per core):
  refs [128, 16384] fp8 : rows 0:64 = ref[b] (c, h*w), 64:128 = ref_align[b]
  tgt  [128, 4096]  bf16: target half, u-stacked: row u*64+c, col f
  o    [128, 4096]  bf16: output half, same u-stacking
Conv PSUM is 4-way tile_position packed: [128, 512] = (phi|g) x (sliceX|sliceY).
2x2 maxpool is two passes: w0-pairs on DVE (PSUM f32 -> SBUF bf16), h0-pairs
on GpSimd.  phi/g biases are added during the post-transpose PSUM->SBUF copy
via a precomputed [128,512] bias tile (bias commutes with maxpool).
"""

import sys

for _p in ("/opt/trn_rl_repo",):
    if _p not in sys.path:
        sys.path.insert(0, _p)

import ml_dtypes
import numpy as np

import concourse.bass as bass
import concourse.mybir as mybir
from concourse import bacc
import concourse.tile as tile
from concourse.masks import make_identity
from concourse.bass_utils import run_bass_kernel_spmd

B, C, IC, H, W = 4, 64, 32, 128, 128
N = H * W            # 16384
NH = N // 2          # spatial positions per core (half batch)
M = N // 4           # 4096 pooled positions per batch
FP32 = mybir.dt.float32
BF16 = mybir.dt.bfloat16
FP8 = mybir.dt.float8e4
SCALE = 16.0         # host scales pgw and biasT by this; /SCALE^2 at gt copy

_CACHED = {}


def _build_program() -> bass.Bass:
    nc = bacc.Bacc("TRN2", target_bir_lowering=False, debug=False)

    refs = nc.dram_tensor("refs", [128, N], FP8, kind="ExternalInput")
    tgt = nc.dram_tensor("tgt", [128, NH // 2], BF16, kind="ExternalInput")
    pgw = nc.dram_tensor("pgw", [128, IC], FP8, kind="ExternalInput")
    wB = nc.dram_tensor("wB", [IC, 2 * C + 1], FP32, kind="ExternalInput")
    wbr = nc.dram_tensor("wbr", [1, C], FP32, kind="ExternalInput")
    biasT = nc.dram_tensor("biasT", [128, 512], BF16, kind="ExternalInput")
    out = nc.dram_tensor("o", [128, NH // 2], BF16, kind="ExternalOutput")

    RCHUNK = 2048        # refs free-dim per DMA (fp8 -> 256 KiB, 2KiB/row)
    TCHUNK = 1024        # target/out free-dim per DMA (bf16 -> 256 KiB)

    with tile.TileContext(nc) as tc, \
         nc.allow_low_precision("bf16/fp8 path well within 2e-2 tolerance"):
        with (
            tc.tile_pool(name="const", bufs=1) as cpool,
            tc.tile_pool(name="p1", bufs=3) as sbT,
            tc.tile_pool(name="small", bufs=2) as sbS,
            tc.tile_pool(name="persist", bufs=1) as pers,
        ):
            # ---- all input DMAs issued up-front on the two earliest-
            # starting engine queues; every destination is persistent so no
            # trigger ever waits on a buffer recycle ----
            pgw_sb = cpool.tile([128, IC], FP8, tag="pgw")
            nc.scalar.dma_start(out=pgw_sb[:], in_=pgw[:])
            refs_t = []
            for k in range(N // RCHUNK):
                t = pers.tile([128, RCHUNK], FP8, tag=f"refs{k}")
                refs_t.append(t)
            for k in (0, 1, 4, 5, 6, 7):
                nc.scalar.dma_start(
                    out=refs_t[k][:], in_=refs[:, k * RCHUNK:(k + 1) * RCHUNK]
                )

            # gpsimd: identity build first (tiny, needed by transposes),
            # then its share of the input stream
            idbf_sb = cpool.tile([128, 128], BF16, tag="identb")
            make_identity(nc, idbf_sb[:])
            id64_sb = cpool.tile([C, C], FP32, tag="ident64")
            make_identity(nc, id64_sb[:])
            one_sb = cpool.tile([1, 1], FP32, tag="one")
            nc.gpsimd.memset(one_sb[:], 1.0)

            for k in (2, 3):
                nc.gpsimd.dma_start(
                    out=refs_t[k][:], in_=refs[:, k * RCHUNK:(k + 1) * RCHUNK]
                )
            wB_sb = cpool.tile([IC, 2 * C + 1], FP32, tag="wB")
            nc.gpsimd.dma_start(out=wB_sb[:], in_=wB[:])
            wbr_sb = cpool.tile([1, C], FP32, tag="wbr")
            nc.gpsimd.dma_start(out=wbr_sb[:], in_=wbr[:])
            biasT_sb = cpool.tile([128, 512], BF16, tag="biasT")
            nc.gpsimd.dma_start(out=biasT_sb[:], in_=biasT[:])
            tgt_tiles = []
            for tq in range((NH // 2) // TCHUNK):
                tt = pers.tile([128, TCHUNK], BF16, tag=f"tgt{tq}")
                tgt_tiles.append(tt)
                nc.gpsimd.dma_start(
                    out=tt[:], in_=tgt[:, tq * TCHUNK:(tq + 1) * TCHUNK]
                )

            thw_sb = wB_sb[:, 0:C]
            wwT_sb = wB_sb[:, C:2 * C]
            thb_sb = wB_sb[:, 2 * C:2 * C + 1]

            # pooled conv outputs (bf16, x16 scaled), 4-way stacked
            pooled = pers.tile([128, 16 * 128], BF16, tag="pooled")
            # transposed pooled (+bias), bf16
            phigT = pers.tile([128, 16 * 128], BF16, tag="phigT")

            # ---- Phase A: fp8 convs + two-pass 2x2 maxpool, with
            # transpose/G/W4 accumulation streamed per 4-chunk group ----
            with tc.tile_pool(name="psA", bufs=3, space="PSUM") as psA, \
                 tc.tile_pool(name="psB", bufs=2, space="PSUM") as psB, \
                 tc.tile_pool(name="psG", bufs=1, space="PSUM") as psG, \
                 tc.tile_pool(name="psW", bufs=1, space="PSUM") as psW:
                w4_ps = psW.tile([128, C], FP32, tag="w4")
                v_ps = psW.tile([IC, 1], FP32, tag="v")
                for cpos in (0, 64):
                    nc.tensor.matmul(
                        w4_ps[cpos:cpos + C, :], id64_sb[:], id64_sb[:],
                        start=True, stop=False,
                        tile_position=(0, cpos), skip_group_check=True,
                    )
                tpp_box = [None]

                def emit_transpose(blk):
                    if blk % 4 == 0:
                        tpp_new = psB.tile([128, 512], BF16, tag="tp")
                        tpp_box[0] = tpp_new
                    bi = blk % 4
                    nc.tensor.matmul(
                        tpp_box[0][:, 128 * bi:128 * (bi + 1)],
                        pooled[:, 128 * blk:128 * (blk + 1)],
                        idbf_sb[:],
                        is_transpose=True, start=True, stop=True,
                        skip_group_check=True,
                    )

                def emit_group_tail(t):
                    # bias'd transposed group -> bf16 SBUF (bias add fused
                    # into the copy), fold into G, then stream partial W4
                    nc.vector.scalar_tensor_tensor(
                        out=phigT[:, 512 * t:512 * (t + 1)],
                        in0=tpp_box[0][:], scalar=1.0, in1=biasT_sb[:],
                        op0=mybir.AluOpType.mult, op1=mybir.AluOpType.add,
                    )
                    g_ps = psG.tile([IC, IC], FP32, tag="G")
                    for c in range(4 * t, 4 * t + 4):
                        b0 = 128 * c
                        nc.tensor.matmul(
                            g_ps[:], phigT[:, b0:b0 + IC],
                            phigT[:, b0 + IC:b0 + 2 * IC],
                            start=(c % 4 == 0), stop=False,
                            skip_group_check=True,
                        )
                        nc.tensor.matmul(
                            g_ps[:], phigT[:, b0 + 2 * IC:b0 + 3 * IC],
                            phigT[:, b0 + 3 * IC:b0 + 4 * IC],
                            start=False, stop=(c % 4 == 3),
                            skip_group_check=True,
                        )
                    gt_sb = sbS.tile([IC, IC], FP32, tag="Gt")
                    nc.scalar.activation(
                        gt_sb[:], g_ps[:],
                        mybir.ActivationFunctionType.Copy,
                        scale=1.0 / (N * SCALE * SCALE),
                    )
                    nc.tensor.matmul(v_ps[:], gt_sb[:], thb_sb,
                                     start=(t == 0), stop=(t == 3),
                                     skip_group_check=True)
                    m2_ps = psG.tile([IC, C], FP32, tag="G")
                    nc.tensor.matmul(m2_ps[:], gt_sb[:], thw_sb,
                                     start=True, stop=True,
                                     skip_group_check=True)
                    m2_sb = sbS.tile([IC, C], FP32, tag="m2sb")
                    nc.scalar.activation(
                        m2_sb[:], m2_ps[:], mybir.ActivationFunctionType.Copy
                    )
                    for cpos in (0, 64):
                        nc.tensor.matmul(
                            w4_ps[cpos:cpos + C, :], m2_sb[:], wwT_sb,
                            start=False, stop=(t == 3 and cpos == 64),
                            tile_position=(0, cpos),
                            skip_group_check=True,
                        )

                for cidx in range(16):
                    rt = refs_t[cidx // 2]
                    j = cidx % 2
                    xs = slice(j * 1024, j * 1024 + 512)
                    ys = slice(j * 1024 + 512, (j + 1) * 1024)
                    cp = psA.tile([128, 512], FP32, tag="conv")
                    nc.tensor.matmul(cp[0:32, :], pgw_sb[0:C, :],
                                     rt[0:C, xs], start=True, stop=True,
                                     tile_position=(0, 0))
                    nc.tensor.matmul(cp[32:64, :], pgw_sb[C:128, :],
                                     rt[C:128, xs], start=True, stop=True,
                                     tile_position=(64, 32))
                    nc.tensor.matmul(cp[64:96, :], pgw_sb[0:C, :],
                                     rt[0:C, ys], start=True, stop=True,
                                     tile_position=(0, 64))
                    nc.tensor.matmul(cp[96:128, :], pgw_sb[C:128, :],
                                     rt[C:128, ys], start=True, stop=True,
                                     tile_position=(64, 96))
                    # pass 1 (DVE): max over w0 pairs, PSUM f32 -> SBUF bf16
                    t1 = sbT.tile([128, 256], BF16, tag="pool1")
                    nc.vector.tensor_reduce(
                        t1[:],
                        cp[:].rearrange("p (a w0) -> p a w0", w0=2),
                        axis=mybir.AxisListType.X, op=mybir.AluOpType.max,
                    )
                    # pass 2 (GpSimd): max over h0 pairs
                    po = pooled[:, cidx * 128:(cidx + 1) * 128]
                    t1v = t1[:].rearrange("p (hp r) -> p hp r", hp=2)
                    nc.vector.tensor_max(
                        po.rearrange("p (hp w) -> p hp w", hp=2),
                        t1v[:, :, 0:64], t1v[:, :, 64:128],
                    )
                    # transpose pipeline runs one 4-chunk group behind
                    prev = cidx - 4
                    if prev >= 0:
                        emit_transpose(prev)
                        if prev % 4 == 3:
                            emit_group_tail(prev // 4)

                for prev in range(12, 16):
                    emit_transpose(prev)
                emit_group_tail(3)

                w4_sb = pers.tile([128, C], BF16, tag="w4sb")
                nc.scalar.activation(
                    w4_sb[:], w4_ps[:], mybir.ActivationFunctionType.Copy
                )
                v_sb = pers.tile([IC, 1], FP32, tag="vsb")
                nc.scalar.activation(
                    v_sb[:], v_ps[:], mybir.ActivationFunctionType.Copy
                )
                # b2 as a per-partition column, duplicated on partitions 64:128
                b2c_ps = psG.tile([128, 1], FP32, tag="G")
                for cpos in (0, 64):
                    nc.tensor.matmul(
                        b2c_ps[cpos:cpos + C, :], wwT_sb, v_sb[:],
                        start=True, stop=False, tile_position=(0, cpos),
                        skip_group_check=True,
                    )
                    nc.tensor.matmul(
                        b2c_ps[cpos:cpos + C, :], wbr_sb[:], one_sb[:, :],
                        start=False, stop=True, tile_position=(0, cpos),
                        skip_group_check=True,
                    )
                b2c_sb = pers.tile([128, 1], FP32, tag="b2csb")
                nc.scalar.activation(
                    b2c_sb[:], b2c_ps[:], mybir.ActivationFunctionType.Copy
                )

            # ---------- Phase D: final 64x64 conv over target (bf16) ----------
            with tc.tile_pool(name="psD", bufs=3, space="PSUM") as psD, \
                 tc.tile_pool(name="outp", bufs=2) as sbO:
                for t in range((NH // 2) // TCHUNK):
                    tt = tgt_tiles[t]
                    ot = sbO.tile([128, TCHUNK], BF16, tag="out")
                    for i in range(TCHUNK // 512):
                        op = psD.tile([128, 512], FP32, tag="od")
                        isl = slice(i * 512, (i + 1) * 512)
                        nc.tensor.matmul(
                            op[0:C, :], w4_sb[0:C, :], tt[0:C, isl],
                            start=True, stop=True, tile_position=(0, 0),
                        )
                        nc.tensor.matmul(
                            op[C:128, :], w4_sb[C:128, :], tt[C:128, isl],
                            start=True, stop=True, tile_position=(64, 64),
                        )
                        # bias-add + bf16 cast, alternating DVE / ACT
                        if i % 2 == 0:
                            nc.vector.tensor_scalar_add(
                                ot[:, isl], op[:], b2c_sb[:]
                            )
                        else:
                            nc.scalar.activation(
                                ot[:, isl], op[:],
                                mybir.ActivationFunctionType.Identity,
                                bias=b2c_sb[:],
                            )
                    nc.sync.dma_start(
                        out=out[:, t * TCHUNK:(t + 1) * TCHUNK], in_=ot[:]
                    )

    nc.compile()
    return nc


def _in_maps(target, ref, ref_align, theta_w, theta_b, phi_w, phi_b,
             g_w, g_b, W_w, W_b):
    f32, bf16 = np.float32, ml_dtypes.bfloat16
    fp8 = ml_dtypes.float8_e4m3
    wBv = np.zeros((IC, 2 * C + 1), dtype=f32)
    wBv[:, 0:C] = theta_w
    wBv[:, C:2 * C] = W_w.T
    wBv[:, 2 * C] = theta_b
    bias512 = np.tile(np.concatenate([phi_b, g_b]) * SCALE, 8).reshape(1, 512)
    common = {
        "pgw": (np.concatenate([phi_w.T, g_w.T], axis=0) * SCALE).astype(fp8),
        "wB": wBv,
        "wbr": W_b.reshape(1, C).astype(f32),
        "biasT": np.broadcast_to(bias512, (128, 512)).astype(bf16),
    }
    maps = []
    for core in range(8):
        b, u = core // 2, core % 2
        refs = np.concatenate(
            [ref[b].reshape(C, N), ref_align[b].reshape(C, N)], axis=0
        ).astype(fp8)
        th = target[b, :, u * (H // 2):(u + 1) * (H // 2), :].reshape(C, NH)
        tgtv = np.concatenate([th[:, :NH // 2], th[:, NH // 2:]], axis=0).astype(bf16)
        maps.append({"refs": np.ascontiguousarray(refs),
                     "tgt": np.ascontiguousarray(tgtv), **common})
    return maps


def _gather(res) -> np.ndarray:
    out = np.empty((B, C, H, W), dtype=np.float32)
    for core in range(8):
        o = np.asarray(res.results[core]["o"]).astype(np.float32)
        half = np.concatenate([o[:C, :], o[C:, :]], axis=1)  # [64, 8192]
        b, u = core // 2, core % 2
        out[b, :, u * (H // 2):(u + 1) * (H // 2), :] = half.reshape(C, H // 2, W)
    return out


def kernel(**inputs) -> np.ndarray:
    if "nc" not in _CACHED:
        _CACHED["nc"] = _build_program()
    nc = _CACHED["nc"]
    maps = _in_maps(**inputs)
    res = run_bass_kernel_spmd(nc, maps, list(range(8)))
    return _gather(res)
